# revision 23
# baseline (speedup 1.0000x reference)
"""Trainium2 Bass kernel for: 3x3 conv (reflect pad) + BatchNorm + LeakyReLU + mask.

Input  x:    (1, 64, 512, 512) f32
       W:    (128, 64, 3, 3)   f32
       gamma/beta/mean/var: (128,) f32
       mask: (1, 128, 512, 512) int32 (0/1)
Output (1, 128, 512, 512) f32

Strategy (8 cores, SPMD):
  - Shard H spatially: core c computes output rows [64c, 64c+64).
  - Even/odd row interleave, single x copy: host reflect-pads x to
    (64, 514, 514); core c's 66-row slab is split by row parity into a
    [128, 33*514] bf16 SBUF image: partition p<64 = channel p at even
    local rows 2k (free col k*WP), partition p>=64 = channel p-64 at odd
    rows 2k+1 (same free col k*WP).
  - One K=128 matmul at free col k then covers taps (dy0,dy1) of even
    output row 2k, and at col k+1 taps (dy1,dy2) of odd row 2k+1. The
    leftover taps (dy2 of even / dy0 of odd) are K=64 matmuls on opposite
    PE row groups + separate PSUM banks -> run concurrently.
    2 output rows = 9 full-rate PE slots (MAC-optimal), x shipped once.
  - PSUM accumulates fp32; epilogue = ACT Lrelu(psum*scale+shift) -> bf16,
    DVE multiply by uint8 mask (bf16, 2x DVE rate); output stored bf16
    (halves store traffic), upcast to f32 on host.
"""

import numpy as np
import ml_dtypes

import concourse.bacc as bacc
import concourse.bass as bass
import concourse.mybir as mybir
import concourse.tile as tile
from concourse.bass_utils import run_bass_kernel_spmd

bf16 = ml_dtypes.bfloat16

N_CORES = 8
C_IN = 64
C_OUT = 128
H = 512
W_IMG = 512
HS = H // N_CORES            # 64 output rows per core
WP = W_IMG + 2               # 514 padded columns
KC = HS // 2 + 1             # 33 k-columns (row pairs) per parity half
FREE = KC * WP               # per-partition free elems of the x image
G = 8                        # mask rows per DMA chunk
LEAK = 0.01
EPS = 1e-5

# store blocks: 4-row tiles, finer at the end to shorten the drain tail
STORE_BLOCKS = [(s, 4) for s in range(0, 56, 4)] + \
               [(s, 2) for s in range(56, 64, 2)]
_Y2BLK = {}
for s, ln in STORE_BLOCKS:
    for y in range(s, s + ln):
        _Y2BLK[y] = (s, ln)

_CACHE = {}
LAST_RESULTS = None          # BassKernelResults of the last run (for test.py)


def _build_program(hw_lrelu: bool = True) -> bass.Bass:
    """hw_lrelu=True uses the ACT engine's native Lrelu (not implemented in
    CoreSim); False uses an Identity + DVE max(z*a, z) fallback."""
    nc = bacc.Bacc("TRN2", target_bir_lowering=False, debug=False,
                   num_devices=N_CORES)
    f32 = mybir.dt.float32
    bf = mybir.dt.bfloat16
    u8 = mybir.dt.uint8

    xs_d = nc.dram_tensor("xs", [128, FREE], bf, kind="ExternalInput")
    wp_d = nc.dram_tensor("wp", [128, 9 * C_OUT], bf, kind="ExternalInput")
    bn_d = nc.dram_tensor("bn", [C_OUT, 2], f32, kind="ExternalInput")
    mk_d = nc.dram_tensor("msk", [C_OUT, HS * W_IMG], u8, kind="ExternalInput")
    out_d = nc.dram_tensor("out", [C_OUT, HS * W_IMG], bf, kind="ExternalOutput")

    with tile.TileContext(nc) as tc:
        with tc.tile_pool(name="const", bufs=1) as cpool, \
             tc.tile_pool(name="zp", bufs=4) as zpool, \
             tc.tile_pool(name="op", bufs=8) as opool, \
             tc.tile_pool(name="ps", bufs=4, space="PSUM") as ppool:

            wt = cpool.tile([128, 9 * C_OUT], bf, name="wt", tag="wt")
            bn = cpool.tile([C_OUT, 2], f32, name="bn_t", tag="bn_t")
            xs = cpool.tile([128, FREE], bf, name="xs_t", tag="xs_t")
            mk = cpool.tile([C_OUT, HS * W_IMG], u8, name="mk_t", tag="mk_t")
            scr = cpool.tile([128, W_IMG], bf, name="scr", tag="scr")

            def load_x(k0, k1, eng=None):
                (eng or nc.sync).dma_start(out=xs[:, k0 * WP:k1 * WP],
                                           in_=xs_d[:, k0 * WP:k1 * WP])

            def load_m(g, eng=None):
                seg = slice(g * G * W_IMG, (g + 1) * G * W_IMG)
                (eng or nc.sync).dma_start(out=mk[:, seg], in_=mk_d[:, seg])

            # ALL loads ride one HWDGE ring (sync/qSPDynamicHW) in strict
            # need order: SDMA engines round-robin between queues at packet
            # granularity, so anything else in flight early delays the
            # critical first x chunk. The ACT ring (scalar) carries only the
            # tiny bn vector + stores; SWDGE carries nothing (shorter
            # gpsimd drain at teardown).
            # only the first 3 weight tiles gate the first matmuls, and those
            # (the dy0/dy1 taps of row 0) read just k-column 0 — load single
            # k-columns first so the PE starts as early as possible
            # weights + bn ride the ACT ring in parallel with the critical
            # first x chunks on the sync ring, so neither gates the other
            nc.scalar.dma_start(out=wt[:, 0:3 * 128], in_=wp_d[:, 0:3 * 128])
            nc.scalar.dma_start(out=wt[:, 3 * 128:], in_=wp_d[:, 3 * 128:])
            nc.scalar.dma_start(out=bn[:], in_=bn_d[:])
            load_x(0, 1, nc.sync)
            load_x(1, 2, nc.sync)
            load_x(2, 4, nc.sync)
            load_m(0, nc.sync)
            load_x(4, 6, nc.sync)
            load_x(6, 8, nc.sync)
            load_m(1, nc.sync)
            load_x(8, 12, nc.sync)
            load_m(2, nc.sync)
            load_x(12, 16, nc.sync)
            load_m(3, nc.sync)
            load_x(16, 20, nc.sync)
            load_m(4, nc.sync)
            load_x(20, 24, nc.sync)
            load_m(5, nc.sync)
            load_x(24, 28, nc.sync)
            load_m(6, nc.sync)
            load_x(28, KC, nc.sync)
            load_m(7, nc.sync)

            # HAM cold-start warmup: the PE powers up at k=4/8 duty and only
            # reaches full rate after ~4us of sustained activity. Run dummy
            # matmuls on a zeroed scratch tile while the first x chunk is
            # still in flight so real matmuls start at full rate.
            nc.vector.memset(scr[:], 0.0)
            ps_w = ppool.tile([C_OUT, 2 * W_IMG], f32, name="ps_w", tag="pst")
            for _ in range(4):
                nc.tensor.matmul(ps_w[:, 0:W_IMG], scr[:, 0:128], scr[:],
                                 start=True, stop=True)

            ot = None

            def epilogue(y, pst):
                # fused 2-row epilogue: y is the even row, pst holds rows
                # y (cols 0:512) and y+1 (cols 512:1024) in adjacent PSUM
                # banks. BN scale/bias are per-output-channel (partition),
                # identical for both rows, so one N=1024 ACT/TT covers the
                # pair and amortizes the 352-cycle ACT instruction overhead.
                s, ln = _Y2BLK[y]
                seg = slice((y - s) * W_IMG, (y - s + 2) * W_IMG)
                mseg = slice(y * W_IMG, (y + 2) * W_IMG)
                if hw_lrelu:
                    nc.scalar.activation(
                        ot[:, seg], pst[:],
                        mybir.ActivationFunctionType.Lrelu,
                        bias=bn[:, 1:2], scale=bn[:, 0:1], alpha=LEAK)
                else:
                    zt = zpool.tile([C_OUT, 2 * W_IMG], f32, name="zt",
                                    tag="zt")
                    nc.scalar.activation(
                        zt[:], pst[:],
                        mybir.ActivationFunctionType.Identity,
                        bias=bn[:, 1:2], scale=bn[:, 0:1])
                    nc.vector.scalar_tensor_tensor(
                        ot[:, seg], zt[:], LEAK, zt[:],
                        op0=mybir.AluOpType.mult, op1=mybir.AluOpType.max)
                nc.vector.tensor_tensor(ot[:, seg], ot[:, seg], mk[:, mseg],
                                        op=mybir.AluOpType.mult)
                if y + 1 == s + ln - 1:
                    d0 = s * W_IMG
                    # early stores ride the ACT ring; later ones are issued
                    # by the sync engine (idle once loads finish) so store
                    # issue never sits between the final ACTs
                    eng = nc.scalar if y < 32 else nc.sync
                    eng.dma_start(out=out_d[:, d0:d0 + ln * W_IMG],
                                  in_=ot[:])

            def epilogue_last(pst):
                # rows 62/63 drain the pipeline: unfused per-row epilogue so
                # ACT(63) overlaps TT(62), and two single-row stores go out
                # on both rings in parallel
                for i, eng in ((0, nc.sync), (1, nc.scalar)):
                    y = 62 + i
                    seg = slice(i * W_IMG, (i + 1) * W_IMG)
                    mseg = slice(y * W_IMG, (y + 1) * W_IMG)
                    if hw_lrelu:
                        nc.scalar.activation(
                            ot[:, seg], pst[:, seg],
                            mybir.ActivationFunctionType.Lrelu,
                            bias=bn[:, 1:2], scale=bn[:, 0:1], alpha=LEAK)
                    else:
                        zl = zpool.tile([C_OUT, W_IMG], f32, name="zl",
                                        tag="ztl")
                        nc.scalar.activation(
                            zl[:], pst[:, seg],
                            mybir.ActivationFunctionType.Identity,
                            bias=bn[:, 1:2], scale=bn[:, 0:1])
                        nc.vector.scalar_tensor_tensor(
                            ot[:, seg], zl[:], LEAK, zl[:],
                            op0=mybir.AluOpType.mult, op1=mybir.AluOpType.max)
                    nc.vector.tensor_tensor(ot[:, seg], ot[:, seg],
                                            mk[:, mseg],
                                            op=mybir.AluOpType.mult)
                    eng.dma_start(out=out_d[:, y * W_IMG:(y + 1) * W_IMG],
                                  in_=ot[:, seg])

            # row pair k -> output rows 2k (ps_a) and 2k+1 (ps_b).
            # wt columns: A(dx)=[Wdy0;Wdy1], B(dx)=[Wdy1;Wdy2],
            #             C(dx)=[Wdy2 (rows 0-63); Wdy0 (rows 64-127)]
            for k in range(HS // 2):
                y = 2 * k
                s, ln = _Y2BLK[y]
                if y == s:
                    ot = opool.tile([C_OUT, ln * W_IMG], bf, name="ot",
                                    tag="ot")
                pst = ppool.tile([C_OUT, 2 * W_IMG], f32, name="pst",
                                 tag="pst")
                for dx in range(3):
                    off_a = k * WP + dx
                    off_b = (k + 1) * WP + dx
                    nc.tensor.matmul(pst[:, 0:W_IMG],
                                     wt[:, dx * 128:dx * 128 + 128],
                                     xs[:, off_a:off_a + W_IMG],
                                     start=(dx == 0), stop=False)
                    nc.tensor.matmul(pst[:, W_IMG:2 * W_IMG],
                                     wt[:, (3 + dx) * 128:(3 + dx) * 128 + 128],
                                     xs[:, off_b:off_b + W_IMG],
                                     start=(dx == 0), stop=False)
                for dx in range(3):
                    cw = slice((6 + dx) * 128, (6 + dx) * 128 + 128)
                    off_a = (k + 1) * WP + dx
                    off_b = k * WP + dx
                    nc.tensor.matmul(pst[:, 0:W_IMG], wt[0:64, cw],
                                     xs[0:64, off_a:off_a + W_IMG],
                                     start=False, stop=(dx == 2))
                    nc.tensor.matmul(pst[:, W_IMG:2 * W_IMG], wt[64:128, cw],
                                     xs[64:128, off_b:off_b + W_IMG],
                                     start=False, stop=(dx == 2))
                if k == HS // 2 - 1:
                    epilogue_last(pst)
                else:
                    epilogue(y, pst)
    nc.compile()
    return nc


def _get_program(hw_lrelu: bool = True) -> bass.Bass:
    key = ("nc", hw_lrelu)
    if key not in _CACHE:
        _CACHE[key] = _build_program(hw_lrelu)
    return _CACHE[key]


def make_in_maps(x, W, gamma, beta, mean, var, mask):
    """Host-side shard/pack of full inputs into per-core in_maps."""
    x = np.asarray(x, np.float32)
    W = np.asarray(W, np.float32)
    gamma = np.asarray(gamma, np.float32)
    beta = np.asarray(beta, np.float32)
    mean = np.asarray(mean, np.float32)
    var = np.asarray(var, np.float32)
    mask = np.asarray(mask)

    xp = np.pad(x[0], ((0, 0), (1, 1), (1, 1)), mode="reflect")   # [64,514,514]
    xpb = xp.astype(bf16)

    # 9 weight tiles [128(K), 128(M)]: A(dx)=[dy0;dy1], B(dx)=[dy1;dy2],
    # C(dx)=[dy2;dy0] (top half: even-row dy2 tap, bottom: odd-row dy0 tap)
    wp9 = np.zeros((9, 128, C_OUT), np.float32)
    wT = [[W[:, :, dy, dx].reshape(C_OUT, C_IN).T for dx in range(3)]
          for dy in range(3)]
    for dx in range(3):
        wp9[dx, 0:64] = wT[0][dx]
        wp9[dx, 64:128] = wT[1][dx]
        wp9[3 + dx, 0:64] = wT[1][dx]
        wp9[3 + dx, 64:128] = wT[2][dx]
        wp9[6 + dx, 0:64] = wT[2][dx]
        wp9[6 + dx, 64:128] = wT[0][dx]
    wp = np.ascontiguousarray(
        wp9.transpose(1, 0, 2).reshape(128, 9 * C_OUT)).astype(bf16)

    inv = 1.0 / np.sqrt(var + EPS)
    bn = np.stack([gamma * inv, beta - mean * gamma * inv],
                  axis=1).astype(np.float32)                      # [128,2]

    m8 = mask[0].astype(np.uint8)                                 # [128,512,512]

    in_maps = []
    for c in range(N_CORES):
        slab = xpb[:, HS * c:HS * c + HS + 2, :]                  # [64,66,514]
        even = np.ascontiguousarray(slab[:, 0::2, :]).reshape(C_IN, FREE)
        odd = np.ascontiguousarray(slab[:, 1::2, :]).reshape(C_IN, FREE)
        xs_c = np.concatenate([even, odd], axis=0)                # [128, FREE]
        mk_c = np.ascontiguousarray(
            m8[:, HS * c:HS * c + HS, :]).reshape(C_OUT, HS * W_IMG)
        in_maps.append(dict(xs=xs_c, wp=wp, bn=bn, msk=mk_c))
    return in_maps


def kernel(x, W, gamma, beta, mean, var, mask, _trace=False):
    global LAST_RESULTS
    nc = _get_program()
    in_maps = make_in_maps(x, W, gamma, beta, mean, var, mask)
    res = run_bass_kernel_spmd(nc, in_maps, list(range(N_CORES)), trace=_trace)
    LAST_RESULTS = res
    out = np.empty((1, C_OUT, H, W_IMG), np.float32)
    for c in range(N_CORES):
        out[0, :, HS * c:HS * c + HS, :] = \
            np.asarray(res.results[c]["out"]).astype(np.float32) \
              .reshape(C_OUT, HS, W_IMG)
    return out
